# revision 10
# baseline (speedup 1.0000x reference)
"""Trainium2 Bass kernel for the masked-correlation colorization net.

Strategy (8 NeuronCores, SPMD):
  - 22 independent (batch, class) "units"; unit u -> core u%8, slot u//8
    (3 slots per core; 2 cores carry a zero-padded dummy slot).
  - Host compacts the ~50%-dense masked pixels per unit (sparse attention):
    comp pixels P (rows) and color pixels Q (softmax columns) are gathered
    and zero-padded to uniform P_pad/Q_pad so all cores run one SPMD program.
  - Device, per unit: center+normalize the [256, n] features in place
    (moments method: nrm2 = sum(sq) - 2*mu.raw + |mu|^2 via PE ones-matmuls,
    rsqrt via ACT sqrt + DVE Newton reciprocal, centering/scaling fused into
    one scalar_tensor_tensor per chunk), corr = cn^T rn with float32r
    matmuls (full PE rate), exp on ACT straight from PSUM (cosine sims are
    bounded, no max pass needed), and a colorize matmul against [img; ones]
    per q-group which yields softmax numerator and denominator together,
    accumulated into SBUF. Host divides and composes the canvas.
"""

import os
import sys

import numpy as np

for _p in ("/opt/trn_rl_repo", os.path.expanduser("~/.axon_site/_ro/trn_rl_repo")):
    if os.path.isdir(_p) and _p not in sys.path:
        sys.path.insert(0, _p)

import concourse.bacc as bacc
import concourse.tile as tile
from concourse import mybir
from concourse.bass_utils import run_bass_kernel_spmd

F32 = mybir.dt.float32
F32R = mybir.dt.float32r
ActF = mybir.ActivationFunctionType
Axis = mybir.AxisListType
Alu = mybir.AluOpType

N_CLASSES = 12
N_CORES = 8
SLOTS = 3      # ceil(2*11 / 8)
QGROUP = 2     # q-tiles per colorize flush group

_PROG_CACHE = {}


def _chunks(total, width):
    out = []
    off = 0
    while off < total:
        w = min(width, total - off)
        out.append((off, w))
        off += w
    return out


def _build_program(P_pad, Q_pad, reps=1):
    key = (P_pad, Q_pad, reps)
    if key in _PROG_CACHE:
        return _PROG_CACHE[key]

    NQ = Q_pad // 128
    p_chunks = _chunks(P_pad, 512)       # colorize N chunks
    NCH = len(p_chunks)
    p_groups = _chunks(P_pad, 1024)      # corr-psum / exp groups
    q_groups = _chunks(Q_pad, 1024)

    nc = bacc.Bacc("TRN2", target_bir_lowering=False, debug=False)

    ins = {}
    outs = {}
    for t in range(SLOTS):
        ins[f"cf{t}"] = nc.dram_tensor(f"cf{t}", [2, 128, P_pad], F32, kind="ExternalInput")
        ins[f"rf{t}"] = nc.dram_tensor(f"rf{t}", [2, 128, Q_pad], F32, kind="ExternalInput")
        ins[f"img{t}"] = nc.dram_tensor(f"img{t}", [128, NQ * 4], F32, kind="ExternalInput")
        ins[f"scl{t}"] = nc.dram_tensor(f"scl{t}", [128, 2], F32, kind="ExternalInput")
        outs[f"out{t}"] = nc.dram_tensor(f"out{t}", [4, P_pad], F32, kind="ExternalOutput")
    ins["ones_in"] = nc.dram_tensor("ones_in", [128, 128], F32, kind="ExternalInput")

    with tile.TileContext(nc) as tc:
        with (
            tc.tile_pool(name="singles", bufs=1) as singles,
            tc.tile_pool(name="feat", bufs=5) as featp,      # raw -> cn in place
            tc.tile_pool(name="sq", bufs=2) as sqp,
            tc.tile_pool(name="nrmb", bufs=1) as nrmbp,
            tc.tile_pool(name="rs", bufs=1) as rsp,
            tc.tile_pool(name="expp", bufs=QGROUP + 1) as expp,
            tc.tile_pool(name="acc", bufs=2) as accp,        # colacc + recip scratch
            tc.tile_pool(name="io", bufs=6) as iop,
            tc.tile_pool(name="psA", bufs=2, space="PSUM") as psA,   # [128,1024] 2 banks x2
            tc.tile_pool(name="psS", bufs=2, space="PSUM") as psS,   # [128,512]  1 bank x2
            tc.tile_pool(name="psP", bufs=2, space="PSUM") as psP,   # [128,512]  1 bank x2
        ):
            ones = singles.tile([128, 128], F32R, tag="ones")
            nc.sync.dma_start(ones[:], ins["ones_in"][:].bitcast(F32R))
            epst = singles.tile([128, 1], F32, tag="eps")
            nc.vector.memset(epst, 1e-30)

            def normalize(dram, width, groups, inv_col):
                """Load [2,128,width] f32r chunks, center+unit-normalize the
                columns in place. Returns the two chunk tiles (= cn)."""
                raws, mus, sqs, nreps, musqs = [], [], [], [], []
                for ch in range(2):
                    rt = featp.tile([128, width], F32R, tag="feat")
                    nc.sync.dma_start(rt[:], dram[ch].bitcast(F32R))
                    raws.append(rt)
                for ch in range(2):
                    musum = iop.tile([128, 1], F32, tag="musum")
                    nc.vector.tensor_reduce(musum[:], raws[ch][:], axis=Axis.X, op=Alu.add)
                    mut = iop.tile([128, 1], F32, tag="mu")
                    nc.vector.tensor_scalar_mul(mut[:], musum[:], inv_col)
                    mus.append(mut)
                    # sq = (raw - mu) * raw
                    st = sqp.tile([128, width], F32R, tag="sq")
                    nc.vector.scalar_tensor_tensor(st[:], raws[ch][:], mut[:],
                                                   raws[ch][:], op0=Alu.subtract,
                                                   op1=Alu.mult)
                    sqs.append(st)
                    # negmurep[k, m] = -mu[k]; musq[k] = mu[k]^2
                    nt = iop.tile([128, 128], F32R, tag="negmurep")
                    nc.vector.tensor_scalar(nt[:], ones[:].bitcast(F32), mut[:], -1.0,
                                            op0=Alu.mult, op1=Alu.mult)
                    nreps.append(nt)
                    mq = iop.tile([128, 1], F32, tag="musq")
                    nc.vector.tensor_mul(mq[:], mut[:], mut[:])
                    musqs.append(mq)
                # |mu|^2 broadcast to [128,1] (+ eps) for the sqrt bias
                psm = psS.tile([128, 512], F32, tag="psS")
                nc.tensor.matmul(psm[:, 0:1], ones[:].bitcast(F32), musqs[0][:], start=True, stop=False)
                nc.tensor.matmul(psm[:, 0:1], ones[:].bitcast(F32), musqs[1][:], start=False, stop=True)
                mu2b = iop.tile([128, 1], F32, tag="mu2b")
                nc.scalar.activation(mu2b[:], psm[:, 0:1], ActF.Identity, bias=epst[:, 0:1])
                # nrm2 = sum(sq) - mu.raw + |mu|^2 (per column), replicated on
                # all partitions via all-ones stationary operand
                nrmt = nrmbp.tile([128, width], F32, tag="nrmb")
                for goff, gw in groups:
                    if gw > 512:
                        ps = psA.tile([128, 1024], F32, tag="psA")
                    else:
                        ps = psS.tile([128, 512], F32, tag="psS")
                    for soff, sw in _chunks(gw, 512):
                        sl = slice(goff + soff, goff + soff + sw)
                        po = slice(soff, soff + sw)
                        nc.tensor.matmul(ps[:, po], ones[:], sqs[0][:, sl], start=True, stop=False)
                        nc.tensor.matmul(ps[:, po], ones[:], sqs[1][:, sl], start=False, stop=False)
                        nc.tensor.matmul(ps[:, po], nreps[0][:], raws[0][:, sl], start=False, stop=False)
                        nc.tensor.matmul(ps[:, po], nreps[1][:], raws[1][:, sl], start=False, stop=True)
                    nc.scalar.activation(nrmt[:, goff:goff + gw], ps[:, :gw],
                                         ActF.Sqrt, bias=mu2b[:, 0:1])
                rst = rsp.tile([128, width], F32, tag="rs")
                scrt = accp.tile([128, width], F32, tag="acc")
                nc.vector.reciprocal_approx_accurate(out=rst[:], in_=nrmt[:], scratch=scrt[:])
                # cn = (raw - mu) * rs, in place
                for ch in range(2):
                    nc.vector.scalar_tensor_tensor(raws[ch][:], raws[ch][:], mus[ch][:],
                                                   rst[:], op0=Alu.subtract, op1=Alu.mult)
                return raws

            for _rep in range(reps):
                for t in range(SLOTS):
                    sclt = iop.tile([128, 2], F32, tag="scl")
                    nc.sync.dma_start(sclt[:], ins[f"scl{t}"][:])
                    imgt = iop.tile([128, NQ * 4], F32R, tag="img")
                    nc.sync.dma_start(imgt[:], ins[f"img{t}"][:].bitcast(F32R))

                    cn = normalize(ins[f"cf{t}"], P_pad, p_groups, sclt[:, 0:1])
                    rn = normalize(ins[f"rf{t}"], Q_pad, q_groups, sclt[:, 1:2])

                    colacc = accp.tile([4, P_pad], F32, tag="acc")
                    nc.vector.memset(colacc[:], 0.0)
                    for qg in range(0, NQ, QGROUP):
                        qts = range(qg, min(qg + QGROUP, NQ))
                        expts = []
                        for qt in qts:
                            qs = slice(qt * 128, (qt + 1) * 128)
                            expt = expp.tile([128, P_pad], F32R, tag="exp")
                            for goff, gw in p_groups:
                                if gw > 512:
                                    ps = psA.tile([128, 1024], F32, tag="psA")
                                else:
                                    ps = psS.tile([128, 512], F32, tag="psS")
                                for soff, sw in _chunks(gw, 512):
                                    sl = slice(goff + soff, goff + soff + sw)
                                    po = slice(soff, soff + sw)
                                    nc.tensor.matmul(ps[:, po], rn[0][:, qs], cn[0][:, sl],
                                                     start=True, stop=False)
                                    nc.tensor.matmul(ps[:, po], rn[1][:, qs], cn[1][:, sl],
                                                     start=False, stop=True)
                                nc.scalar.activation(expt[:, goff:goff + gw], ps[:, :gw],
                                                     ActF.Exp)
                            expts.append(expt)
                        for j, (poff, pw) in enumerate(p_chunks):
                            part = psP.tile([128, 512], F32, tag="psP")
                            for qi, qt in enumerate(qts):
                                nc.tensor.matmul(part[0:4, :pw],
                                                 imgt[:, qt * 4:qt * 4 + 4],
                                                 expts[qi][:, poff:poff + pw],
                                                 start=(qi == 0), stop=(qi == len(qts) - 1),
                                                 skip_group_check=True)
                            nc.vector.tensor_add(colacc[:, poff:poff + pw],
                                                 colacc[:, poff:poff + pw],
                                                 part[0:4, :pw])
                    nc.sync.dma_start(outs[f"out{t}"][:], colacc[:])

    nc.compile()
    _PROG_CACHE[key] = (nc, NQ, p_chunks)
    return _PROG_CACHE[key]


def _prepare(inputs):
    """Host-side gather/pad. Returns (in_maps, units, P_pad, Q_pad, ...)."""
    cfeat = np.asarray(inputs["comp_C_feature"], np.float32)
    rfeat = np.asarray(inputs["color_C_feature"], np.float32)
    img = np.asarray(inputs["color_C_img"], np.float32)
    cmask = np.asarray(inputs["comp_O_mask"])
    lmask = np.asarray(inputs["color_O_mask"])
    size = int(np.asarray(inputs["size"]))
    B, C, H, W = cfeat.shape
    S = size * size

    cfeat = cfeat.reshape(B, C, S)
    rfeat = rfeat.reshape(B, C, S)
    img = img.reshape(B, 3, S)
    cm = cmask.reshape(B, N_CLASSES, S) > 0
    lm = lmask.reshape(B, N_CLASSES, S) > 0

    units = []
    for b in range(B):
        for k in range(1, N_CLASSES):
            cidx = np.nonzero(cm[b, k])[0]
            lidx = np.nonzero(lm[b, k])[0]
            n_c, n_l = len(cidx), len(lidx)
            units.append({
                "b": b, "k": k, "n_c": n_c, "n_l": n_l,
                "valid": (n_c > 1) and (n_l > 1),
                "cidx": cidx, "lidx": lidx,
            })

    max_nc = max([u["n_c"] for u in units if u["valid"]] + [1])
    max_nl = max([u["n_l"] for u in units if u["valid"]] + [1])
    P_pad = max(256, -(-max_nc // 256) * 256)   # 512-chunks + >=256 tail
    Q_pad = max(128, -(-max_nl // 128) * 128)
    while Q_pad % 1024 > 512:                    # psum group remainder <= 512
        Q_pad += 128
    NQ = Q_pad // 128

    in_maps = []
    for c in range(N_CORES):
        m = {}
        for t in range(SLOTS):
            u = t * N_CORES + c
            cf_buf = np.zeros((2, 128, P_pad), np.float32)
            rf_buf = np.zeros((2, 128, Q_pad), np.float32)
            img_buf = np.zeros((128, NQ * 4), np.float32)
            scl_buf = np.ones((128, 2), np.float32)
            if u < len(units) and units[u]["valid"]:
                uu = units[u]
                b, n_c, n_l = uu["b"], uu["n_c"], uu["n_l"]
                cf_buf[:, :, :n_c] = cfeat[b][:, uu["cidx"]].reshape(2, 128, n_c)
                rf_buf[:, :, :n_l] = rfeat[b][:, uu["lidx"]].reshape(2, 128, n_l)
                img4 = np.zeros((4, Q_pad), np.float32)
                img4[:3, :n_l] = img[b][:, uu["lidx"]]
                img4[3, :n_l] = 1.0
                img_buf[:] = img4.T.reshape(NQ, 128, 4).transpose(1, 0, 2).reshape(128, NQ * 4)
                scl_buf[:, 0] = 1.0 / max(n_c, 1)
                scl_buf[:, 1] = 1.0 / max(n_l, 1)
            m[f"cf{t}"] = cf_buf
            m[f"rf{t}"] = rf_buf
            m[f"img{t}"] = img_buf
            m[f"scl{t}"] = scl_buf
        m["ones_in"] = np.ones((128, 128), np.float32)
        in_maps.append(m)
    return in_maps, units, P_pad, Q_pad, img, B, S, H, W


def kernel(**inputs):
    in_maps, units, P_pad, Q_pad, img, B, S, H, W = _prepare(inputs)
    nc, NQ, p_chunks = _build_program(P_pad, Q_pad)
    res = run_bass_kernel_spmd(nc, in_maps, list(range(N_CORES)))

    canvas = -np.ones((B, 3, S), np.float32)
    for u, uu in enumerate(units):
        if not uu["valid"]:
            continue
        core, t = u % N_CORES, u // N_CORES
        cols = np.asarray(res.results[core][f"out{t}"], np.float32)
        n_c = uu["n_c"]
        vals = cols[:3, :n_c] / cols[3:4, :n_c]
        canvas[uu["b"]][:, uu["cidx"]] = vals
    return canvas.reshape(B, 3, H, W).astype(np.float32)
